# Initial kernel scaffold
#
"""Trainium2 Bass kernel for nn_DistanceProbe.

Computes, for batch [B=8, S=2048, H=768] and proj [H=768, R=768]:
    t  = batch @ proj                      # [B, S, R]
    d2 = relu(||t_i||^2 + ||t_j||^2 - 2 t_i . t_j)   # [B, S, S]

Sharding: data-parallel over B across the 8 NeuronCores (one batch
element per core). Each core receives its batch slice pre-transposed
(xT = batch[b].T, [H, S]) so the contraction dim H lands on SBUF
partitions without any on-device transpose.

Per-core device algorithm (all matmuls in float32r = full-rate fp32):
  1. tT[r, s]   = sum_h proj[h, r] * xT[h, s]        (PE, K=H)
  2. sq[s]      = sum_r tT[r, s]^2                   (DVE square + ones-matmul)
  3. psum[i, j] = sum_r tT[r, i] * tT[r, j]          (PE, K=R)
  4. out[i, j]  = relu(-2*psum + sq_j + sq_i)        (DVE stt + ACT relu w/ bias)
"""

import numpy as np

import concourse.bass as bass
import concourse.tile as tile
from concourse import mybir
from concourse.bass_utils import run_bass_kernel_spmd

B, S, H, R = 8, 2048, 768, 768
N_CORES = 8
P = 128          # SBUF partitions
NC_ = 512        # matmul moving free dim (one PSUM bank of fp32)
HT = H // P      # 6  k-tiles over H
RT = R // P      # 6  k-tiles over R
IT = S // P      # 16 output row tiles
SC = S // NC_    # 4  512-wide column chunks

F32 = mybir.dt.float32


def build_nc(mm_dtype=mybir.dt.float32r):
    nc = bass.Bass("TRN2", target_bir_lowering=False, debug=False,
                   num_devices=N_CORES)

    xT_d = nc.dram_tensor("xT", [H, S], mm_dtype, kind="ExternalInput")
    pj_d = nc.dram_tensor("proj", [H, R], mm_dtype, kind="ExternalInput")
    out_d = nc.dram_tensor("out", [S, S], F32, kind="ExternalOutput")

    with tile.TileContext(nc) as tc:
        with tc.tile_pool(name="persist", bufs=1) as sb, \
             tc.tile_pool(name="stage", bufs=4) as stg, \
             tc.tile_pool(name="pmm", bufs=2, space="PSUM") as pmm, \
             tc.tile_pool(name="psq", bufs=1, space="PSUM") as psq, \
             tc.tile_pool(name="pd", bufs=5, space="PSUM") as pdp:

            xT_sb = [sb.tile([P, S], mm_dtype, name=f"xT{i}", tag=f"xT{i}")
                     for i in range(HT)]
            pj_sb = [sb.tile([P, R], mm_dtype, name=f"pj{i}", tag=f"pj{i}")
                     for i in range(HT)]
            tT_sb = [sb.tile([P, S], mm_dtype, name=f"tT{i}", tag=f"tT{i}")
                     for i in range(RT)]
            sqj = sb.tile([P, S], F32, name="sqj", tag="sqj")
            sqrow = sb.tile([1, S], mm_dtype, name="sqrow", tag="sqrow")
            sqcol = sb.tile([P, IT], F32, name="sqcol", tag="sqcol")
            ones_col = sb.tile([P, 1], mm_dtype, name="ones_col", tag="onc")
            ones_row = sb.tile([1, P], mm_dtype, name="ones_row", tag="onr")

            nc.vector.memset(ones_col[:], 1.0)
            nc.vector.memset(ones_row[:], 1.0)

            # ---- loads: proj first (every matmul group needs all of it) ----
            for ht in range(HT):
                nc.sync.dma_start(pj_sb[ht][:], pj_d[ht * P:(ht + 1) * P, :])
            for sc in range(SC):
                for ht in range(HT):
                    nc.sync.dma_start(
                        xT_sb[ht][:, sc * NC_:(sc + 1) * NC_],
                        xT_d[ht * P:(ht + 1) * P, sc * NC_:(sc + 1) * NC_])

            # ---- phase B: tT = projT @ x  (tT[r, s]) ----
            for sc in range(SC):
                for rt in range(RT):
                    pt = pmm.tile([P, NC_], F32, name="pt", tag="pt")
                    for ht in range(HT):
                        nc.tensor.matmul(
                            pt[:],
                            pj_sb[ht][:, rt * P:(rt + 1) * P],
                            xT_sb[ht][:, sc * NC_:(sc + 1) * NC_],
                            start=(ht == 0), stop=(ht == HT - 1))
                    nc.scalar.copy(tT_sb[rt][:, sc * NC_:(sc + 1) * NC_], pt[:])

            # ---- squares (into the dead xT tiles) + sq row reduction ----
            for rt in range(RT):
                nc.vector.tensor_mul(xT_sb[rt][:], tT_sb[rt][:], tT_sb[rt][:])
            for sc in range(SC):
                sq_ps = psq.tile([1, NC_], F32, name="sq_ps", tag="sq")
                for rt in range(RT):
                    nc.tensor.matmul(
                        sq_ps[:], ones_col[:],
                        xT_sb[rt][:, sc * NC_:(sc + 1) * NC_],
                        start=(rt == 0), stop=(rt == RT - 1))
                nc.vector.tensor_copy(sqrow[0:1, sc * NC_:(sc + 1) * NC_],
                                      sq_ps[:])

            # ---- sq broadcast across partitions (ones_row^T @ sqrow) ----
            for sc in range(SC):
                bc = pmm.tile([P, NC_], F32, name="bc", tag="pt")
                nc.tensor.matmul(bc[:], ones_row[:],
                                 sqrow[0:1, sc * NC_:(sc + 1) * NC_],
                                 start=True, stop=True)
                nc.vector.tensor_copy(sqj[:, sc * NC_:(sc + 1) * NC_], bc[:])

            # ---- sq column form: 16x PE transpose of [1,128] slices ----
            for it in range(IT):
                tp = pmm.tile([P, 1], F32, name="tp", tag="pt")
                nc.tensor.transpose(tp[:], sqrow[0:1, it * P:(it + 1) * P],
                                    ones_row[0:1, 0:1])
                nc.vector.tensor_copy(sqcol[:, it:it + 1], tp[:])

            # ---- phase D: dots + fused epilogue ----
            for it in range(IT):
                for jc in range(SC):
                    pd = pdp.tile([P, NC_], F32, name="pd", tag="pd")
                    for rt in range(RT):
                        nc.tensor.matmul(
                            pd[:],
                            tT_sb[rt][:, it * P:(it + 1) * P],
                            tT_sb[rt][:, jc * NC_:(jc + 1) * NC_],
                            start=(rt == 0), stop=(rt == RT - 1))
                    st = stg.tile([P, NC_], F32, name="st", tag="st")
                    nc.vector.scalar_tensor_tensor(
                        st[:], pd[:], -2.0, sqj[:, jc * NC_:(jc + 1) * NC_],
                        mybir.AluOpType.mult, mybir.AluOpType.add)
                    st2 = stg.tile([P, NC_], F32, name="st2", tag="st2")
                    nc.scalar.activation(st2[:], st[:],
                                         mybir.ActivationFunctionType.Relu,
                                         bias=sqcol[:, it:it + 1], scale=1.0)
                    nc.sync.dma_start(
                        out_d[it * P:(it + 1) * P, jc * NC_:(jc + 1) * NC_],
                        st2[:])

    nc.compile()
    return nc


_NC_CACHE = {}


def get_nc(mm_dtype=mybir.dt.float32r):
    key = str(mm_dtype)
    if key not in _NC_CACHE:
        _NC_CACHE[key] = build_nc(mm_dtype)
    return _NC_CACHE[key]


def make_in_maps(batch, proj):
    proj = np.ascontiguousarray(proj, dtype=np.float32)
    return [
        {"xT": np.ascontiguousarray(batch[b].T, dtype=np.float32),
         "proj": proj}
        for b in range(B)
    ]


def kernel(batch, proj):
    assert batch.shape == (B, S, H) and proj.shape == (H, R)
    nc = get_nc()
    in_maps = make_in_maps(batch, proj)
    res = run_bass_kernel_spmd(nc, in_maps, core_ids=list(range(N_CORES)))
    out = np.stack([res.results[b]["out"] for b in range(B)], axis=0)
    return out.astype(np.float32, copy=False)


# revision 9
# speedup vs baseline: 20.5730x; 20.5730x over previous
"""Trainium2 Bass kernel for nn_DistanceProbe.

Computes, for batch [B=8, S=2048, H=768] and proj [H=768, R=768]:
    t  = batch @ proj                      # [B, S, R]
    d2 = relu(||t_i||^2 + ||t_j||^2 - 2 t_i . t_j)   # [B, S, S]

Sharding: data-parallel over B across the 8 NeuronCores (one batch
element per core). Each core receives its batch slice pre-transposed
(xT = batch[b].T, [H, S]) so the contraction dim H lands on SBUF
partitions without any on-device transpose.

Per-core device algorithm (all matmuls in float32r = full-rate fp32):
  1. tT[r, s]   = sum_h proj[h, r] * xT[h, s]        (PE, K=H)
  2. sq[s]      = sum_r tT[r, s]^2                   (DVE square + ones-matmul)
  3. psum[i, j] = sum_r tT[r, i] * tT[r, j]          (PE, K=R)
  4. out[i, j]  = relu(-2*psum + sq_j + sq_i)        (DVE stt + ACT relu w/ bias)

`reps` repeats the whole body inside one NEFF (used by test.py to
measure steady-state HW time by differencing two rep counts).
"""

import numpy as np

import concourse.bass as bass
import concourse.tile as tile
from concourse import bacc
from concourse import mybir
from concourse.bass_utils import run_bass_kernel_spmd

B, S, H, R = 8, 2048, 768, 768
N_CORES = 8
P = 128          # SBUF partitions
NC_ = 512        # matmul moving free dim (one PSUM bank of fp32)
HT = H // P      # 6  k-tiles over H
RT = R // P      # 6  k-tiles over R
IT = S // P      # 16 output row tiles
SC = S // NC_    # 4  512-wide column chunks

F32 = mybir.dt.float32


def build_nc(mm_dtype=mybir.dt.float32r, reps=1):
    nc = bacc.Bacc("TRN2", target_bir_lowering=False, debug=False,
                   num_devices=N_CORES)

    xT_d = nc.dram_tensor("xT", [H, S], mm_dtype, kind="ExternalInput")
    pj_d = nc.dram_tensor("proj", [H, R], mm_dtype, kind="ExternalInput")
    out_d = nc.dram_tensor("out", [S, S], F32, kind="ExternalOutput")

    with tile.TileContext(nc) as tc:
        with tc.tile_pool(name="persist", bufs=1) as sb, \
             tc.tile_pool(name="stage", bufs=4) as stg, \
             tc.tile_pool(name="pmm", bufs=2, space="PSUM") as pmm, \
             tc.tile_pool(name="psq", bufs=1, space="PSUM") as psq, \
             tc.tile_pool(name="pd", bufs=5, space="PSUM") as pdp:

            xT_sb = [sb.tile([P, S], mm_dtype, name=f"xT{i}", tag=f"xT{i}")
                     for i in range(HT)]
            pj_sb = [sb.tile([P, R], mm_dtype, name=f"pj{i}", tag=f"pj{i}")
                     for i in range(HT)]
            tT_sb = [sb.tile([P, S], mm_dtype, name=f"tT{i}", tag=f"tT{i}")
                     for i in range(RT)]
            sqj = sb.tile([P, S], F32, name="sqj", tag="sqj")
            sqrow = sb.tile([1, S], mm_dtype, name="sqrow", tag="sqrow")
            sqrow_f = sb.tile([1, S], F32, name="sqrow_f", tag="sqrowf")
            sqcol = sb.tile([P, IT], F32, name="sqcol", tag="sqcol")
            ones_col = sb.tile([P, 1], mm_dtype, name="ones_col", tag="onc")
            ones_row = sb.tile([1, P], mm_dtype, name="ones_row", tag="onr")
            onesf_col = sb.tile([P, 1], F32, name="onesf_col", tag="onfc")
            onesf_row = sb.tile([1, P], F32, name="onesf_row", tag="onfr")

            nc.vector.memset(onesf_col[:], 1.0)
            nc.vector.memset(onesf_row[:], 1.0)
            nc.vector.tensor_copy(ones_col[:], onesf_col[:])
            nc.vector.tensor_copy(ones_row[:], onesf_row[:])

            def emit_body():
                # loads: proj first (every matmul group needs all of it)
                for ht in range(HT):
                    nc.sync.dma_start(pj_sb[ht][:],
                                      pj_d[ht * P:(ht + 1) * P, :])
                for sc in range(SC):
                    for ht in range(HT):
                        nc.sync.dma_start(
                            xT_sb[ht][:, sc * NC_:(sc + 1) * NC_],
                            xT_d[ht * P:(ht + 1) * P, sc * NC_:(sc + 1) * NC_])

                # phase B: tT = projT @ x  (tT[r, s])
                for sc in range(SC):
                    for rt in range(RT):
                        pt = pmm.tile([P, NC_], F32, name="pt", tag="pt")
                        for ht in range(HT):
                            nc.tensor.matmul(
                                pt[:],
                                pj_sb[ht][:, rt * P:(rt + 1) * P],
                                xT_sb[ht][:, sc * NC_:(sc + 1) * NC_],
                                start=(ht == 0), stop=(ht == HT - 1))
                        nc.scalar.copy(tT_sb[rt][:, sc * NC_:(sc + 1) * NC_],
                                       pt[:])

                # squares (into the dead xT tiles) + sq row reduction
                for rt in range(RT):
                    nc.vector.tensor_mul(xT_sb[rt][:], tT_sb[rt][:],
                                         tT_sb[rt][:])
                for sc in range(SC):
                    sq_ps = psq.tile([1, NC_], F32, name="sq_ps", tag="sq")
                    for rt in range(RT):
                        nc.tensor.matmul(
                            sq_ps[:], ones_col[:],
                            xT_sb[rt][:, sc * NC_:(sc + 1) * NC_],
                            start=(rt == 0), stop=(rt == RT - 1))
                    nc.vector.tensor_copy(sqrow[0:1, sc * NC_:(sc + 1) * NC_],
                                          sq_ps[:])
                    nc.vector.tensor_copy(
                        sqrow_f[0:1, sc * NC_:(sc + 1) * NC_], sq_ps[:])

                # sq broadcast across partitions (ones_row^T @ sqrow)
                for sc in range(SC):
                    bc = pmm.tile([P, NC_], F32, name="bc", tag="pt")
                    nc.tensor.matmul(bc[:], ones_row[:],
                                     sqrow[0:1, sc * NC_:(sc + 1) * NC_],
                                     start=True, stop=True)
                    nc.vector.tensor_copy(sqj[:, sc * NC_:(sc + 1) * NC_],
                                          bc[:])

                # sq column form: 16x PE transpose of [1,128] slices
                for it in range(IT):
                    tp = pmm.tile([P, 1], F32, name="tp", tag="pt")
                    nc.tensor.transpose(tp[:],
                                        sqrow_f[0:1, it * P:(it + 1) * P],
                                        onesf_row[0:1, 0:1])
                    nc.vector.tensor_copy(sqcol[:, it:it + 1], tp[:])

                # phase D: dots + fused epilogue
                for it in range(IT):
                    for jc in range(SC):
                        pd = pdp.tile([P, NC_], F32, name="pd", tag="pd")
                        for rt in range(RT):
                            nc.tensor.matmul(
                                pd[:],
                                tT_sb[rt][:, it * P:(it + 1) * P],
                                tT_sb[rt][:, jc * NC_:(jc + 1) * NC_],
                                start=(rt == 0), stop=(rt == RT - 1))
                        st = stg.tile([P, NC_], F32, name="st", tag="st")
                        nc.vector.scalar_tensor_tensor(
                            st[:], pd[:], -2.0,
                            sqj[:, jc * NC_:(jc + 1) * NC_],
                            mybir.AluOpType.mult, mybir.AluOpType.add)
                        st2 = stg.tile([P, NC_], F32, name="st2", tag="st2")
                        nc.scalar.activation(
                            st2[:], st[:], mybir.ActivationFunctionType.Relu,
                            bias=sqcol[:, it:it + 1], scale=1.0)
                        nc.sync.dma_start(
                            out_d[it * P:(it + 1) * P, jc * NC_:(jc + 1) * NC_],
                            st2[:])

            for _ in range(reps):
                emit_body()

    nc.finalize()
    return nc


_NC_CACHE = {}


def get_nc(mm_dtype=mybir.dt.float32r, reps=1):
    key = (str(mm_dtype), reps)
    if key not in _NC_CACHE:
        _NC_CACHE[key] = build_nc(mm_dtype, reps)
    return _NC_CACHE[key]


def make_in_maps(batch, proj):
    proj = np.ascontiguousarray(proj, dtype=np.float32)
    return [
        {"xT": np.ascontiguousarray(batch[b].T, dtype=np.float32),
         "proj": proj}
        for b in range(B)
    ]


def kernel(batch, proj):
    assert batch.shape == (B, S, H) and proj.shape == (H, R)
    nc = get_nc()
    in_maps = make_in_maps(batch, proj)
    res = run_bass_kernel_spmd(nc, in_maps, core_ids=list(range(N_CORES)))
    out = np.stack([res.results[b]["out"] for b in range(B)], axis=0)
    return out.astype(np.float32, copy=False)
